# revision 7
# baseline (speedup 1.0000x reference)
"""DirectedEdgeConv Trainium2 kernel, 8-core SPMD — v2 (descriptor-free).

out[e] = prelu(x[e] @ Wself^T + b + T_in[src[e]] + T_out[dst[e]], 0.2)
  T_in  = scatter_mean(x, dst) @ Win^T    (bias folded into prelu's bias)
  T_out = scatter_mean(x, src) @ Wout^T

Sharding: edges assigned to core own(src). Per core:
  A1: stream dst-owner-grouped edges (dense KA buckets per 128-node
      block, bf16), one-hot PE scatter -> T_in[own nodes], finalized
      (x inv_cnt, @Win^T) and kept RESIDENT in SBUF (bf16, [n,d] blocks).
      No collective needed for T_in.
  A2: same for src-grouped edges -> T_out[own], finalized, transposed
      to [d,n] fp32 and written per-block to DRAM; one AllGather makes
      the full [d, 50176] table; relayout DMAs stage one half
      ([128, 25088] fp32, 98KB/partition) in SBUF at a time.
  C:  edges sorted by (dst_half, src_block, dst), padded per
      (half, block) to 128-multiples so the tile->block map is uniform
      across cores (SPMD).  Per 1536-edge supertile: one gpsimd
      ap_gather (SBUF->SBUF, 8 Q7 cores, no DMA descriptors) expands
      T_out^T columns; per 128-edge tile: ones-bcast matmul + is_equal
      builds the transposed one-hot, then 2 accumulated matmuls give
      psum[d,e] = Wself^T-mm(xT) + T_in-block-mm(ohT); DVE adds the
      gathered T_out^T; ACT applies Prelu(+bias).  y is written bf16,
      transposed [d, e]; the host de-permutes.
"""

import sys

sys.path.insert(0, "/opt/trn_rl_repo")

import numpy as np
import ml_dtypes

import concourse.bacc as bacc
import concourse.bass as bass
import concourse.mybir as mybir
import concourse.tile as tile
from concourse import library_config
from concourse.bass_utils import run_bass_kernel_spmd
from concourse.masks import make_identity

BF = ml_dtypes.bfloat16

P = 128
D = 128
C = 8
E = 600000
N = 50000
NPC = N // C            # 6250
NB = (NPC + P - 1) // P  # 49
NBP = NB * P            # 6272
TROWS = C * NBP         # 50176
HALFT = TROWS // 2      # 25088
KC = 24
SUP = P * KC            # 3072

F32 = mybir.dt.float32
BF16 = mybir.dt.bfloat16
I16 = mybir.dt.int16

PRELU = mybir.ActivationFunctionType.Prelu


def build_kernel(KAB_d, OFF_d, KAB_s, OFF_s, NSUP_H, BLK):
    """NSUP_H: (nsup_half0, nsup_half1). BLK: tuple of per-tile block ids,
    len = 12*(nsup0+nsup1), uniform across cores."""
    NSUP = NSUP_H[0] + NSUP_H[1]
    SIDX = SUP // 16

    nc = bacc.Bacc(None, target_bir_lowering=False, debug=False)

    # ---- I/O ----
    SUMD, SUMS = int(OFF_d[-1]), int(OFF_s[-1])
    KAMAX = max(max(KAB_d), max(KAB_s))
    agat_d = nc.dram_tensor("agat_d", [P, SUMD * D], BF16, kind="ExternalInput")
    va_d = nc.dram_tensor("va_d", [P, SUMD], F32, kind="ExternalInput")
    agat_s = nc.dram_tensor("agat_s", [P, SUMS * D], BF16, kind="ExternalInput")
    va_s = nc.dram_tensor("va_s", [P, SUMS], F32, kind="ExternalInput")
    invc_d = nc.dram_tensor("invc_d", [P, NB], F32, kind="ExternalInput")
    invc_s = nc.dram_tensor("invc_s", [P, NB], F32, kind="ExternalInput")
    xT_d = nc.dram_tensor("xT", [NSUP, P, SUP], BF16, kind="ExternalInput")
    srcv_d = nc.dram_tensor("srcv", [NSUP, 1, SUP], BF16, kind="ExternalInput")
    gidx_d = nc.dram_tensor("gidx", [NSUP, P, SIDX], I16, kind="ExternalInput")
    win = nc.dram_tensor("win", [D, D], BF16, kind="ExternalInput")      # W_in_w.T
    wout = nc.dram_tensor("wout", [D, D], BF16, kind="ExternalInput")    # W_out_w.T
    wselfT = nc.dram_tensor("wselfT", [D, D], BF16, kind="ExternalInput")  # W_self_w.T
    bbcol = nc.dram_tensor("bbcol", [P, 1], F32, kind="ExternalInput")   # W_self_b col
    iota_in = nc.dram_tensor("iota", [P, P], BF16, kind="ExternalInput")
    iotac_in = nc.dram_tensor("iotac", [P, 1], F32, kind="ExternalInput")
    y = nc.dram_tensor("y", [NSUP * P, SUP], BF16, kind="ExternalOutput")

    with tile.TileContext(nc) as tc:
        with (
            tc.tile_pool(name="const", bufs=1) as cpool,
            tc.tile_pool(name="sbuf", bufs=4) as pool,
            tc.tile_pool(name="small", bufs=4) as spool,
            tc.tile_pool(name="psumA", bufs=1, space="PSUM") as psum,
            tc.tile_pool(name="psumC", bufs=2, space="PSUM") as psumc,
            tc.tile_pool(name="dram", bufs=1, space="DRAM") as dram,
        ):
            nc.gpsimd.load_library(library_config.mlp)
            # ---- constants ----
            ident = cpool.tile([P, P], BF16)
            make_identity(nc, ident[:])
            iota_t = cpool.tile([P, P], BF16)
            nc.sync.dma_start(out=iota_t[:], in_=iota_in[:])
            iota_c = cpool.tile([P, 1], F32)
            nc.sync.dma_start(out=iota_c[:], in_=iotac_in[:])
            ones_t = cpool.tile([1, P], BF16)
            nc.vector.memset(ones_t[:], 1.0)
            win_t = cpool.tile([D, D], BF16)
            nc.sync.dma_start(out=win_t[:], in_=win[:])
            wout_t = cpool.tile([D, D], BF16)
            nc.sync.dma_start(out=wout_t[:], in_=wout[:])
            wselfT_t = cpool.tile([D, D], BF16)
            nc.sync.dma_start(out=wselfT_t[:], in_=wselfT[:])
            bb_t = cpool.tile([P, 1], F32)
            nc.sync.dma_start(out=bb_t[:], in_=bbcol[:])
            invc_d_t = cpool.tile([P, NB], F32)
            nc.sync.dma_start(out=invc_d_t[:], in_=invc_d[:])
            invc_s_t = cpool.tile([P, NB], F32)
            nc.sync.dma_start(out=invc_s_t[:], in_=invc_s[:])
            # resident T_in blocks [n_local, dout] bf16, one tile per block
            tinb = [cpool.tile([P, D], BF16, name=f"tinb{b}") for b in range(NB)]
            cc_in_a = dram.tile([25 * P, D], BF16)
            cc_in_b = dram.tile([24 * P, D], BF16)
            cc_out_a = dram.tile([C * 25 * P, D], BF16)
            cc_out_b = dram.tile([C * 24 * P, D], BF16)

            # ---- phase A ----
            def phase_a(agat, va, KAB, OFF, invc_t, w_t, to_tin, dmae):
                for b in range(NB):
                    KA, off = int(KAB[b]), int(OFF[b])
                    valt = spool.tile([P, KAMAX], F32, tag="aval")
                    dmae.dma_start(out=valt[:, :KA], in_=va[:, off : off + KA])
                    gat = pool.tile([P, KAMAX * D], BF16, tag="agather")
                    dmae.dma_start(
                        out=gat[:, : KA * D],
                        in_=agat[:, off * D : (off + KA) * D],
                    )
                    ps = psum.tile([P, D], F32, tag="pA")
                    for j in range(KA):
                        oh = spool.tile([P, P], BF16, tag="oh")
                        nc.vector.tensor_scalar(
                            oh[:], iota_t[:], valt[:, j : j + 1], None,
                            mybir.AluOpType.is_equal,
                        )
                        nc.tensor.matmul(
                            ps[:], oh[:], gat[:, j * D : (j + 1) * D],
                            start=(j == 0), stop=(j == KA - 1),
                        )
                    means = spool.tile([P, D], BF16, tag="means")
                    nc.vector.tensor_scalar(
                        means[:], ps[:], invc_t[:, b : b + 1], None,
                        mybir.AluOpType.mult,
                    )
                    pst = psum.tile([P, D], BF16, tag="pB")
                    nc.tensor.transpose(pst[:], means[:], ident[:])
                    meansT = spool.tile([P, D], BF16, tag="meansT")
                    nc.scalar.copy(out=meansT[:], in_=pst[:])
                    psT = psum.tile([P, D], F32, tag="pC")
                    nc.tensor.matmul(psT[:], meansT[:], w_t[:], start=True, stop=True)
                    if to_tin:
                        # [node, dout] -> resident bf16 block
                        nc.scalar.copy(out=tinb[b][:], in_=psT[:])
                    else:
                        # [node, dout] bf16 rows staged to DRAM for AllGather
                        tt = spool.tile([P, D], BF16, tag="tt")
                        nc.scalar.copy(out=tt[:], in_=psT[:])
                        if b < 25:
                            nc.sync.dma_start(
                                out=cc_in_a[b * P : (b + 1) * P, :], in_=tt[:]
                            )
                        else:
                            nc.sync.dma_start(
                                out=cc_in_b[(b - 25) * P : (b - 24) * P, :],
                                in_=tt[:],
                            )

            phase_a(agat_s, va_s, KAB_s, OFF_s, invc_s_t, wout_t, False, nc.sync)
            nc.gpsimd.collective_compute(
                "AllGather", mybir.AluOpType.bypass,
                replica_groups=[list(range(C))],
                ins=[cc_in_a.opt()], outs=[cc_out_a.opt()],
            )
            phase_a(agat_d, va_d, KAB_d, OFF_d, invc_d_t, win_t, True, nc.scalar)

            # ---- phase C ----
            s_global = 0
            for h in range(2):
                tbl_half = (cc_out_a if h == 0 else cc_out_b)[:, :]
                for i_h in range(NSUP_H[h]):
                    s = s_global
                    s_global += 1
                    if h == 0 and i_h == 2:
                        nc.gpsimd.collective_compute(
                            "AllGather", mybir.AluOpType.bypass,
                            replica_groups=[list(range(C))],
                            ins=[cc_in_b.opt()], outs=[cc_out_b.opt()],
                        )
                    xT_t = pool.tile([P, SUP], BF16, tag="xT")
                    nc.sync.dma_start(out=xT_t[:], in_=xT_d[s])
                    srcv_t = spool.tile([1, SUP], BF16, tag="srcv")
                    nc.sync.dma_start(out=srcv_t[:], in_=srcv_d[s])
                    gix = spool.tile([P, SIDX], I16, tag="gix")
                    nc.sync.dma_start(out=gix[:], in_=gidx_d[s])
                    go = pool.tile([P, SUP], BF16, tag="go")
                    nc.gpsimd.dma_gather(
                        out_ap=go[:].rearrange("p (a e) -> p a e", a=1),
                        in_ap=tbl_half,
                        idxs_ap=gix[:],
                        num_idxs=SUP, num_idxs_reg=SUP, elem_size=D,
                        transpose=True, single_packet=False,
                    )
                    yo = pool.tile([P, SUP], BF16, tag="yo")
                    W = 4 * D  # 512-wide groups
                    for g in range(KC // 4):
                        slg = slice(g * W, (g + 1) * W)
                        psB = psumc.tile([P, W], F32, tag="qB")
                        nc.tensor.matmul(
                            psB[:], ones_t[:], srcv_t[:, slg], start=True, stop=True
                        )
                        ohT = spool.tile([P, W], BF16, tag="ohT")
                        nc.vector.tensor_scalar(
                            ohT[:], psB[:], iota_c[:, 0:1], None,
                            mybir.AluOpType.is_equal,
                        )
                        psA = psumc.tile([P, W], F32, tag="qA", bufs=3)
                        nc.tensor.matmul(
                            psA[:], wselfT_t[:], xT_t[:, slg], start=True, stop=False
                        )
                        for k in range(4):
                            t = g * 4 + k
                            blk = BLK[s * KC + t]
                            sk = slice(k * D, (k + 1) * D)
                            nc.tensor.matmul(
                                psA[:, sk], tinb[blk][:],
                                ohT[:, sk], start=False, stop=True,
                                skip_group_check=True,
                            )
                        st = spool.tile([P, W], BF16, tag="st")
                        nc.vector.tensor_add(st[:], psA[:], go[:, slg])
                        nc.scalar.activation(
                            yo[:, slg], st[:], PRELU,
                            bias=bb_t[:, 0:1], scale=1.0, alpha=0.2,
                        )
                    nc.sync.dma_start(out=y[s * P : (s + 1) * P, :], in_=yo[:])

    nc.compile()
    return nc


def prepare_inputs(edge_attr, edge_index, W_self_w, W_self_b, W_in_w, W_out_w):
    edge_attr = np.ascontiguousarray(edge_attr, dtype=np.float32)
    src = np.asarray(edge_index[0], dtype=np.int64)
    dst = np.asarray(edge_index[1], dtype=np.int64)

    win = np.ascontiguousarray(np.asarray(W_in_w, np.float32).T).astype(BF)
    wout = np.ascontiguousarray(np.asarray(W_out_w, np.float32).T).astype(BF)
    wselfT = np.ascontiguousarray(np.asarray(W_self_w, np.float32).T).astype(BF)
    bbcol = np.asarray(W_self_b, np.float32).reshape(P, 1)
    iota = np.tile(np.arange(P, dtype=np.float32)[None, :], (P, 1)).astype(BF)
    iotac = np.arange(P, dtype=np.float32).reshape(P, 1)

    # ---- phase A dense buckets (same scheme as v1) ----
    def build_a(node_of_edge):
        core = node_of_edge // NPC
        local = node_of_edge - core * NPC
        inblk = (local & 127).astype(np.float32)
        blk = (local >> 7).astype(np.int64)
        key = (core * NB + blk).astype(np.int64)
        order = np.argsort(key, kind="stable")
        cnts = np.bincount(key, minlength=C * NB)
        KAb = np.maximum(
            1, np.ceil(cnts.reshape(C, NB).max(axis=0) / P).astype(np.int64)
        )
        offs = np.zeros(NB + 1, dtype=np.int64)
        np.cumsum(KAb, out=offs[1:])
        SUM = int(offs[-1])
        starts = np.zeros(C * NB, dtype=np.int64)
        np.cumsum(cnts[:-1], out=starts[1:])
        pos = np.arange(E, dtype=np.int64) - starts[key[order]]
        b_o, c_o = blk[order], core[order]
        kae = KAb[b_o]
        p_o = pos // kae
        j_o = pos - p_o * kae
        flat = (c_o * P + p_o) * SUM + offs[b_o] + j_o
        agat = np.zeros((C * P * SUM, D), dtype=BF)
        agat[flat] = edge_attr[order].astype(BF)
        agat = agat.reshape(C, P, SUM * D)
        va = np.full((C * P * SUM), -1.0, dtype=np.float32)
        va[flat] = inblk[order]
        va = va.reshape(C, P, SUM)
        cnt_node = np.bincount(node_of_edge, minlength=N).astype(np.float32)
        inv = 1.0 / np.maximum(cnt_node, 1.0)
        inv_pad = np.zeros((C, NBP), dtype=np.float32)
        inv_pad[:, :NPC] = inv.reshape(C, NPC)
        invc = np.ascontiguousarray(inv_pad.reshape(C, NB, P).transpose(0, 2, 1))
        return tuple(KAb), tuple(offs), agat, va, invc

    KA_dst, OFF_dst, agat_d, va_d, invc_d = build_a(dst)
    KA_src, OFF_src, agat_s, va_s, invc_s = build_a(src)

    # ---- phase C structure (uniform across cores) ----
    HLOC = 25 * P  # 3200: halves split per-core locals at block 25
    core_d = dst // NPC
    loc_d = dst - core_d * NPC
    half_e = (loc_d >= HLOC).astype(np.int64)
    rows_half = np.where(
        half_e == 0, core_d * HLOC + loc_d,
        core_d * (NBP - HLOC) + (loc_d - HLOC),
    )
    core_e = src // NPC
    src_loc = src - core_e * NPC
    blk_e = src_loc >> 7

    # per-core, per (half, block) counts -> uniform tile counts
    cnt = np.zeros((C, 2, NB), dtype=np.int64)
    for c in range(C):
        m = core_e == c
        np.add.at(cnt[c], (half_e[m], blk_e[m]), 1)
    maxcnt = cnt.max(axis=0)  # [2, NB]
    T_hb = np.maximum(1, np.ceil(maxcnt / P).astype(np.int64))  # tiles per (h,b)
    tiles_h = T_hb.sum(axis=1)
    NSUP_H = tuple(int(np.ceil(th / KC)) for th in tiles_h)
    NSUP = NSUP_H[0] + NSUP_H[1]
    NT = NSUP * KC
    # per-tile block ids (uniform): concat [b]*T_hb then pad to supertile mult
    BLK = []
    tile_base = np.zeros((2, NB), dtype=np.int64)  # first tile of (h,b)
    tb = 0
    for h in range(2):
        for b in range(NB):
            tile_base[h, b] = tb
            BLK.extend([b] * int(T_hb[h, b]))
            tb += int(T_hb[h, b])
        pad = NSUP_H[h] * KC - (tb - (0 if h == 0 else NSUP_H[0] * KC))
        BLK.extend([0] * pad)
        tb += pad
    BLK = tuple(BLK)
    assert len(BLK) == NT

    def wrap_idx(vals):
        S = len(vals) // 16
        t = np.zeros((16, S), dtype=np.int16)
        t[np.arange(len(vals)) % 16, np.arange(len(vals)) // 16] = vals.astype(
            np.int16
        )
        return np.tile(t, (8, 1))

    in_maps = []
    posts = []
    for c in range(C):
        m = core_e == c
        eids = np.nonzero(m)[0]
        # sort by (half, block, dst)
        okey = (half_e[eids] * NB + blk_e[eids]) * (2 * N) + dst[eids]
        order = np.argsort(okey, kind="stable")
        eids = eids[order]
        # slot assignment: per (h,b) segment starts at tile_base*P
        slot_edge = np.full(NT * P, -1, dtype=np.int64)
        pos = 0
        for h in range(2):
            for b in range(NB):
                k = int(cnt[c, h, b])
                base = int(tile_base[h, b]) * P
                slot_edge[base : base + k] = eids[pos : pos + k]
                pos += k
        assert pos == len(eids)
        valid = slot_edge >= 0
        ge = np.where(valid, slot_edge, 0)
        xs = np.where(valid[:, None], edge_attr[ge], 0).astype(BF)
        # [NT*P, D] -> [NSUP, P(d), SUP(cols)]
        xT = np.ascontiguousarray(
            xs.reshape(NSUP, SUP, D).transpose(0, 2, 1)
        )
        sv = np.where(valid, src_loc[ge] & 127, -1).astype(np.float32).astype(BF)
        sv = np.ascontiguousarray(sv.reshape(NSUP, 1, SUP))
        gi = np.where(valid, rows_half[ge], 0)
        gidx = np.stack(
            [wrap_idx(gi[s * SUP : (s + 1) * SUP]) for s in range(NSUP)]
        )
        in_maps.append(
            dict(
                agat_d=agat_d[c], va_d=va_d[c], agat_s=agat_s[c], va_s=va_s[c],
                invc_d=invc_d[c], invc_s=invc_s[c],
                xT=xT, srcv=sv, gidx=gidx,
                win=win, wout=wout, wselfT=wselfT, bbcol=bbcol,
                iota=iota, iotac=iotac,
            )
        )
        posts.append((slot_edge, valid))

    def postprocess(results):
        full = np.empty((E, D), dtype=np.float32)
        for c in range(C):
            slot_edge, valid = posts[c]
            yv = np.asarray(results[c]["y"]).astype(np.float32)
            yv = yv.reshape(NSUP, P, SUP).transpose(0, 2, 1).reshape(NT * P, D)
            full[slot_edge[valid]] = yv[valid]
        return full

    params = (KA_dst, OFF_dst, KA_src, OFF_src, NSUP_H, BLK)
    return params, in_maps, postprocess


_NC_CACHE = {}


def run(inputs, trace=False, trace_kwargs=None):
    params, in_maps, post = prepare_inputs(
        inputs["edge_attr"],
        inputs["edge_index"],
        inputs["W_self_w"],
        inputs["W_self_b"],
        inputs["W_in_w"],
        inputs["W_out_w"],
    )
    key = params
    if key not in _NC_CACHE:
        _NC_CACHE[key] = build_kernel(*params)
    nc = _NC_CACHE[key]
    kw = {}
    if trace:
        kw["trace"] = True
        if trace_kwargs:
            kw.update(trace_kwargs)
    res = run_bass_kernel_spmd(nc, in_maps, core_ids=list(range(C)), **kw)
    return post(res.results), res


def kernel(**inputs) -> np.ndarray:
    out, _ = run(inputs)
    return out.astype(np.float32)
